# revision 35
# baseline (speedup 1.0000x reference)
"""Distributed GIN (3-layer) kernel for 8 TRN2 NeuronCores.

Sharding: nodes (and their incident in-edges) are partitioned across the 8
cores; each core keeps its node-feature shard resident in SBUF (transposed,
fp32).  Between layers the per-core shards are AllGathered into a bf16
"gather table" in DRAM; neighbor aggregation is a dma_gather (256B/edge) +
one-hot matmul segment-sum on the tensor engine.  MLP weights are replicated.
Per-graph pooled sums are computed per-core on the PE and reduced on the host.
"""

import math

import ml_dtypes
import numpy as np

N_NODES = 100000
N_EDGES = 1600000
F = 128              # feature dim (= hidden dim)
L = 3                # GIN layers
G = 64               # graphs
NC = 8               # NeuronCores
PER = N_NODES // NC  # 12500 real nodes per core
BLK = 128
NBLK = (PER + BLK - 1) // BLK          # 98 node blocks per core
PERP = NBLK * BLK                      # 12544 padded rows per core
TROWS = NC * PERP                      # 100352 table rows
NQ = 4
QS = TROWS // NQ                       # 25088 rows per quadrant (< int16 max)
RBLK = 4                               # dst blocks per PSUM round (4 PSUM banks)
NR = (NBLK + RBLK - 1) // RBLK         # 25 rounds
GRBLK = 4                              # dst blocks per gather call (1 round)
NGR = (NBLK + GRBLK - 1) // GRBLK      # 13 gather super-rounds
MLP_CHUNK = 512
AG_SPLITS = (13 * MLP_CHUNK, 22 * MLP_CHUNK)   # piece boundaries (per-core rows)
AGS = AG_SPLITS[0]
DEBUG_DUMPS = False
REPS = 1

_F32 = np.float32
_BF16 = ml_dtypes.bfloat16


def _ag_pieces():
    bounds = (0,) + AG_SPLITS + (PERP,)
    return [(bounds[i], bounds[i + 1]) for i in range(len(bounds) - 1)]


def _table_row(node):
    """Global node id -> row in the piece-major all-gathered table:
    pieces of every core concatenated piece-major."""
    c = node // PER
    r = node % PER
    out = np.zeros_like(c)
    base = 0
    for lo, hi in _ag_pieces():
        w = hi - lo
        sel = (r >= lo) & (r < hi)
        out = np.where(sel, base + c * w + (r - lo), out)
        base += NC * w
    return out


def _round_blocks(r):
    b0 = r * RBLK
    return range(b0, min(b0 + RBLK, NBLK))


def _gr_blocks(g):
    b0 = g * GRBLK
    return range(b0, min(b0 + GRBLK, NBLK))


def _build_structures(edge_index, batch):
    """Host-side preprocessing: per-core padded edge-slot streams.

    Returns a dict with per-core planes plus the shared capacity map.
    """
    src = np.asarray(edge_index[0], dtype=np.int64)
    dst = np.asarray(edge_index[1], dtype=np.int64)
    srow = _table_row(src)
    q_of_edge = srow // QS
    qloc = (srow % QS).astype(np.int16)

    core_of_edge = dst // PER
    dst_local = dst % PER
    b_of_edge = dst_local // BLK

    # per-core (q, b) counts -> shared capacities (in 128-edge tiles)
    counts = np.zeros((NC, NQ, NBLK), dtype=np.int64)
    np.add.at(counts, (core_of_edge, q_of_edge, b_of_edge), 1)
    K = np.ceil(counts.max(axis=0) / BLK).astype(np.int64)  # [NQ, NBLK]

    # static slot-stream layout: gather super-rounds -> q -> blocks
    group_off = np.zeros((NQ, NBLK), dtype=np.int64)
    tile_of = {}      # (q, b) -> global tile index of the block's first tile
    call_meta = []    # per (g, q): (slot_offset, n_slots)
    off = 0
    for g in range(NGR):
        for q in range(NQ):
            call_off = off
            for b in _gr_blocks(g):
                group_off[q, b] = off
                tile_of[(q, b)] = off // BLK
                off += int(K[q, b]) * BLK
            call_meta.append((g, q, call_off, off - call_off))
    tot_slots = off
    assert tot_slots % BLK == 0
    tot_tiles = tot_slots // BLK

    # per-(round, quadrant) column offsets into the host-built one-hot plane
    oh_off = {}
    oh_cols = 0
    for r in range(NR):
        rblocks = [b for b in _round_blocks(r) if b < NBLK]
        for q in range(NQ):
            Tr = int(sum(K[q, b] for b in rblocks))
            oh_off[(r, q)] = oh_cols
            oh_cols += 128 * Tr
    assert oh_cols == 128 * tot_tiles

    idx_planes = []
    oh_planes = []
    ohg_planes = []
    bg_planes = []
    xT_dummy = None
    js = np.arange(BLK, dtype=np.float32)
    for c in range(NC):
        sel = core_of_edge == c
        eq = q_of_edge[sel]
        eb = b_of_edge[sel]
        eloc = qloc[sel]
        edrel = (dst_local[sel] - eb * BLK).astype(np.float32)

        idx = np.zeros(tot_slots, dtype=np.int16)
        drel = np.full(tot_slots, -1.0, dtype=np.float32)
        # order edges by (q, b); place each (q, b) run at its static offset
        order = np.lexsort((eb, eq))
        eq, eb, eloc, edrel = eq[order], eb[order], eloc[order], edrel[order]
        gid = eq * NBLK + eb
        run_starts = np.zeros(NQ * NBLK + 1, dtype=np.int64)
        np.add.at(run_starts, gid + 1, 1)
        run_starts = np.cumsum(run_starts)
        flat_off = group_off.reshape(-1)
        pos = flat_off[gid] + (np.arange(gid.size) - run_starts[gid])
        idx[pos] = eloc
        drel[pos] = edrel

        idx_plane = np.tile(idx.reshape(-1, 16).T, (8, 1))          # [128, tot/16]
        drel_plane = drel.reshape(-1, BLK).T                        # [128, tot_tiles]
        idx_planes.append(np.ascontiguousarray(idx_plane))

        # one-hot plane: per (r, q) group of Tr tiles, col = j*Tr + (t - t0);
        # oh[p, j, t] = 1 iff slot p of tile t targets dst row j of its block
        ohp = np.zeros((128, oh_cols), dtype=_BF16)
        for r in range(NR):
            rblocks = [b for b in _round_blocks(r) if b < NBLK]
            for q in range(NQ):
                Tr = int(sum(K[q, b] for b in rblocks))
                if Tr == 0:
                    continue
                t0 = tile_of[(q, rblocks[0])]
                c0 = oh_off[(r, q)]
                sub = drel_plane[:, t0:t0 + Tr]                      # [128, Tr]
                oh3 = (sub[:, None, :] == js[None, :, None])
                ohp[:, c0:c0 + 128 * Tr] = oh3.reshape(128, 128 * Tr).astype(_BF16)
        oh_planes.append(ohp)

        bgfull = np.full(NBLK * BLK, -1.0, dtype=np.float32)
        bgfull[:PER] = np.asarray(batch[c * PER:(c + 1) * PER], dtype=np.float32)
        bgp = bgfull.reshape(NBLK, BLK).T                            # [128, NBLK]
        bg_planes.append(np.ascontiguousarray(bgp))
        ohg = (bgp[:, :, None] == np.arange(G, dtype=np.float32)[None, None, :])
        ohg_planes.append(np.ascontiguousarray(ohg.reshape(128, NBLK * G).astype(_BF16)))

    OHT = 0
    for r in range(NR):
        for q in range(NQ):
            OHT = max(OHT, int(sum(K[q, b] for b in _round_blocks(r))))

    return {
        "K": K,
        "OHT": OHT,
        "tile_of": tile_of,
        "call_meta": call_meta,
        "tot_slots": tot_slots,
        "tot_tiles": tot_tiles,
        "oh_off": oh_off,
        "oh_cols": oh_cols,
        "idx_planes": idx_planes,
        "oh_planes": oh_planes,
        "ohg_planes": ohg_planes,
        "bg_planes": bg_planes,
    }


def _build_program(struct, eps_vals):
    import concourse.bacc as bacc
    import concourse.mybir as mybir
    from concourse import tile

    dt = mybir.dt
    AF = mybir.ActivationFunctionType
    OP = mybir.AluOpType

    tot_slots = struct["tot_slots"]
    tot_tiles = struct["tot_tiles"]
    call_meta = struct["call_meta"]

    nc = bacc.Bacc("TRN2", target_bir_lowering=False, num_swdge_queues=4)

    # ---- kernel parameters (per-core values via in_maps) ----
    xT_p = nc.declare_dram_parameter("xT", [F, PERP], dt.float32, isOutput=False)
    idx_p = nc.declare_dram_parameter("idx", [128, tot_slots // 16], dt.int16, isOutput=False)
    ohp_p = nc.declare_dram_parameter("ohp", [128, struct["oh_cols"]], dt.bfloat16, isOutput=False)
    ohg_p = nc.declare_dram_parameter("ohg", [128, NBLK * G], dt.bfloat16, isOutput=False)
    iota_p = nc.declare_dram_parameter("iota", [128, 128], dt.bfloat16, isOutput=False)
    bg_p = nc.declare_dram_parameter("bg", [128, NBLK], dt.float32, isOutput=False)
    OHT = struct["OHT"]
    ident_p = nc.declare_dram_parameter("ident", [128, 128], dt.bfloat16, isOutput=False)
    epsI_p = nc.declare_dram_parameter("epsI", [L, 128, 128], dt.bfloat16, isOutput=False)
    Wp1_p = nc.declare_dram_parameter("Wp1", [F, F], dt.float32, isOutput=False)
    bp1_p = nc.declare_dram_parameter("bp1", [F, 1], dt.float32, isOutput=False)
    Wp2_p = nc.declare_dram_parameter("Wp2", [F, F], dt.float32, isOutput=False)
    bp2_p = nc.declare_dram_parameter("bp2", [F, 1], dt.float32, isOutput=False)
    W1_p = nc.declare_dram_parameter("W1s", [L, F, F], dt.float32, isOutput=False)
    b1_p = nc.declare_dram_parameter("b1s", [L, F, 1], dt.float32, isOutput=False)
    W2_p = nc.declare_dram_parameter("W2s", [L, F, F], dt.float32, isOutput=False)
    b2_p = nc.declare_dram_parameter("b2s", [L, F, 1], dt.float32, isOutput=False)
    out_p = nc.declare_dram_parameter("out", [G, L * F], dt.float32, isOutput=True)
    if DEBUG_DUMPS:
        dbg_hrow = nc.declare_dram_parameter("dbg_hrow", [128, 128], dt.bfloat16, isOutput=True)
        dbg_ohg = nc.declare_dram_parameter("dbg_ohg", [128, G], dt.bfloat16, isOutput=True)

    # ---- internal DRAM ----
    ag_in = nc.dram_tensor("ag_in", [PERP, F], dt.bfloat16)
    tables = [
        nc.dram_tensor(f"table{l}", [TROWS, F], dt.bfloat16, addr_space="Shared")
        for l in range(L)
    ]

    with tile.TileContext(nc) as tc:
        with (
            tc.tile_pool(name="const", bufs=1) as cpool,
            tc.tile_pool(name="ht", bufs=1) as hpool,
            tc.tile_pool(name="gath", bufs=15) as gpool,
            tc.tile_pool(name="idxp", bufs=14) as ipool,
            tc.tile_pool(name="oh", bufs=13) as ohpool,
            tc.tile_pool(name="zz", bufs=2) as zpool,
            tc.tile_pool(name="emit", bufs=4) as epool,
            tc.tile_pool(name="psag", bufs=4, space="PSUM") as psag,
            tc.tile_pool(name="psmlp", bufs=2, space="PSUM") as psmlp,
            tc.tile_pool(name="pstr", bufs=1, space="PSUM") as pstr,
            tc.tile_pool(name="pspool", bufs=1, space="PSUM") as pspool,
        ):
            # ---- load constants / weights ----
            ident_sb = cpool.tile([128, 128], dt.bfloat16, tag="ident")
            nc.sync.dma_start(ident_sb[:], ident_p[:])
            epsI = []
            for l in range(L):
                t = cpool.tile([128, 128], dt.bfloat16, tag=f"epsI{l}")
                nc.sync.dma_start(t[:], epsI_p[l][:])
                epsI.append(t)
            ohg_sb = cpool.tile([128, NBLK, G], dt.bfloat16, tag="ohg")
            nc.sync.dma_start(ohg_sb[:].rearrange("p a b -> p (a b)"), ohg_p[:])
            iota_sb = cpool.tile([128, 128], dt.bfloat16, tag="iota")
            nc.sync.dma_start(iota_sb[:], iota_p[:])
            bg_sb = cpool.tile([128, NBLK], dt.float32, tag="bg")
            nc.sync.dma_start(bg_sb[:], bg_p[:])

            def _load_w(tag, pslice):
                t = cpool.tile([F, F], dt.float32, tag=tag)
                nc.sync.dma_start(t[:], pslice)
                return t

            def _load_b(tag, pslice):
                t = cpool.tile([F, 1], dt.float32, tag=tag)
                nc.sync.dma_start(t[:], pslice)
                return t

            Wp1 = _load_w("Wp1", Wp1_p[:])
            Wp2 = _load_w("Wp2", Wp2_p[:])
            bp1 = _load_b("bp1", bp1_p[:])
            bp2 = _load_b("bp2", bp2_p[:])
            W1 = [_load_w(f"W1_{l}", W1_p[l][:]) for l in range(L)]
            W2 = [_load_w(f"W2_{l}", W2_p[l][:]) for l in range(L)]
            b1 = [_load_b(f"b1_{l}", b1_p[l][:]) for l in range(L)]
            b2 = [_load_b(f"b2_{l}", b2_p[l][:]) for l in range(L)]

            hT = hpool.tile([F, PERP], dt.bfloat16, tag="hT")

            def _ag_piece(l_next, piece):
                pieces = _ag_pieces()
                lo, hi = pieces[piece]
                base = NC * sum(h - l2 for l2, h in pieces[:piece])
                nc.gpsimd.collective_compute(
                    "AllGather", OP.bypass,
                    replica_groups=[list(range(NC))],
                    ins=[ag_in[lo:hi, :]],
                    outs=[tables[l_next][base:base + NC * (hi - lo), :]],
                )

            pool_psums = []

            def _emit_block(b, layer_out):
                """Transpose block b of hT; DMA to ag_in (if a table is
                still needed) and accumulate pooling (if layer_out >= 1)."""
                ptr = pstr.tile([128, 128], dt.bfloat16, tag="tr")
                nc.tensor.transpose(ptr[:], hT[:, b * BLK:(b + 1) * BLK], ident_sb[:])
                hrow = epool.tile([128, 128], dt.bfloat16, tag="hrow")
                nc.scalar.activation(hrow[:], ptr[:], AF.Copy)
                if layer_out < L:
                    nc.sync.dma_start(ag_in[b * BLK:(b + 1) * BLK, :], hrow[:])
                if layer_out >= 1:
                    ohg = ohpool.tile([128, G], dt.bfloat16, tag="ohgb")
                    nc.sync.dma_start(ohg[:], ohg_p[:, b * G:(b + 1) * G])
                    nc.tensor.matmul(
                        pool_psums[layer_out - 1][:],
                        ohg[:],
                        hrow[:],
                        start=(b == 0),
                        stop=(b == NBLK - 1),
                        skip_group_check=True,
                    )

            for _rep in range(REPS):
                # ---- pre-MLP: hT = relu(relu(x Wp1 + bp1) Wp2 + bp2), transposed ----
                o = 0
                while o < PERP:
                    cw = min(MLP_CHUNK, PERP - o)
                    xc = zpool.tile([F, MLP_CHUNK], dt.float32, tag="xc")
                    nc.sync.dma_start(xc[:, :cw], xT_p[:, o:o + cw])
                    p1 = psmlp.tile([F, MLP_CHUNK], dt.float32, tag="mlp")
                    nc.tensor.matmul(p1[:, :cw], Wp1[:], xc[:, :cw])
                    t1 = zpool.tile([F, MLP_CHUNK], dt.float32, tag="t1")
                    nc.scalar.activation(t1[:, :cw], p1[:, :cw], AF.Relu, bias=bp1[:])
                    p2 = psmlp.tile([F, MLP_CHUNK], dt.float32, tag="mlp")
                    nc.tensor.matmul(p2[:, :cw], Wp2[:], t1[:, :cw])
                    nc.scalar.activation(hT[:, o:o + cw], p2[:, :cw], AF.Relu, bias=bp2[:])
                    for k in range(cw // BLK):
                        _emit_block(o // BLK + k, 0)
                    if o + cw in AG_SPLITS:
                        _ag_piece(0, AG_SPLITS.index(o + cw))
                    o += cw
                _ag_piece(0, len(AG_SPLITS))


                # ---- GIN layers ----
                for l in range(L):
                    pool_psums.append(pspool.tile([G, F], dt.float32, tag="pool", name=f"poolp{l}"))

                    # PSUM accumulation groups are bank-granular: each block gets
                    # its own [F, BLK] psum tile (padded to one bank) and all of
                    # its matmuls are consecutive.  Gathers stay q-major per round
                    # (big calls); matmuls consume the SBUF buffers block-major.
                    K = struct["K"]

                    def _mlp_chunk(o, cw, agg_of):
                        z = zpool.tile([F, MLP_CHUNK], dt.float32, tag="z",
                                       name=f"z_l{l}_o{o}")
                        for k in range(cw // BLK):
                            b = o // BLK + k
                            nc.scalar.activation(
                                z[:, k * BLK:(k + 1) * BLK], agg_of[b][:], AF.Copy,
                            )
                        p1 = psmlp.tile([F, MLP_CHUNK], dt.float32, tag="mlp",
                                        name=f"p1_l{l}_o{o}")
                        nc.tensor.matmul(p1[:, :cw], W1[l][:], z[:, :cw])
                        t1 = zpool.tile([F, MLP_CHUNK], dt.float32, tag="t1",
                                        name=f"t1_l{l}_o{o}")
                        nc.scalar.activation(t1[:, :cw], p1[:, :cw], AF.Relu, bias=b1[l][:])
                        p2 = psmlp.tile([F, MLP_CHUNK], dt.float32, tag="mlp",
                                        name=f"p2_l{l}_o{o}")
                        nc.tensor.matmul(p2[:, :cw], W2[l][:], t1[:, :cw])
                        nc.scalar.activation(
                            hT[:, o:o + cw], p2[:, :cw], AF.Identity, bias=b2[l][:]
                        )
                        for k in range(cw // BLK):
                            _emit_block(o // BLK + k, l + 1)

                    tile_of = struct["tile_of"]
                    vis_done = {b: 0 for b in range(NBLK)}
                    PF = 3  # gather groups of prefetch ahead of consumption
                    round_state = {}

                    def _issue_round(g):
                        # gathers (one per quadrant) + idx + oh loads for round g
                        gts = {}
                        for (gg, q, call_off, n_slots) in call_meta:
                            if gg != g:
                                continue
                            # every (g, q) call must be issued to keep the
                            # DMASW-lane <-> SWDGE-queue binding aligned
                            assert n_slots > 0, (g, q)
                            T = n_slots // BLK
                            idxs = ipool.tile([128, n_slots // 16], dt.int16,
                                              tag="idxs", name=f"idxs_l{l}_g{g}_q{q}")
                            nc.sync.dma_start(
                                idxs[:], idx_p[:, call_off // 16:(call_off + n_slots) // 16]
                            )
                            gt = gpool.tile([128, T, 128], dt.bfloat16, tag="gt",
                                            name=f"gt_l{l}_g{g}_q{q}")
                            nc.gpsimd.dma_gather(
                                gt[:],
                                tables[l][q * QS:(q + 1) * QS, :],
                                idxs[:],
                                n_slots,
                                n_slots,
                                F,
                                single_packet=False,
                                queue_num=q,
                            )
                            gts[q] = (gt, call_off // BLK)
                        for r in range(g * GRBLK // RBLK,
                                       min((g + 1) * GRBLK, NBLK + RBLK - 1) // RBLK):
                            rblocks = [b for b in _round_blocks(r) if b < NBLK]
                            if not rblocks:
                                continue
                            ohs = {}
                            for q in range(NQ):
                                t0 = tile_of.get((q, rblocks[0]))
                                Tr = sum(int(K[q, b]) for b in rblocks)
                                if Tr == 0 or q not in gts:
                                    continue
                                oh = ohpool.tile([128, 128, Tr], dt.bfloat16, tag="oh",
                                                 name=f"oh_l{l}_r{r}_q{q}")
                                c0 = struct["oh_off"][(r, q)]
                                nc.sync.dma_start(
                                    oh[:].rearrange("p a b -> p (a b)"),
                                    ohp_p[:, c0:c0 + 128 * Tr],
                                )
                                ohs[q] = (oh, t0)
                            round_state[r] = (gts, ohs, rblocks)

                    def _consume_round(r):
                        gts, ohs, rblocks = round_state.pop(r)
                        # per-block PSUM accumulators, one full bank each;
                        # chain opens with the (1+eps)*h self-term matmul
                        agg_of = {}
                        for b in rblocks:
                            agg_of[b] = psag.tile([F, BLK], dt.float32, tag="agg",
                                                  name=f"agg_l{l}_b{b}")
                            nvis = int(K[:, b].sum())
                            nc.tensor.matmul(
                                agg_of[b][:],
                                epsI[l][:],
                                hT[:, b * BLK:(b + 1) * BLK],
                                start=True,
                                stop=(nvis == 0),
                                skip_group_check=True,
                            )
                        for q in range(NQ):
                            if q not in ohs:
                                continue
                            oh, t0 = ohs[q]
                            gt, c0 = gts[q]
                            for b in rblocks:
                                nvis = int(K[:, b].sum())
                                bt = tile_of[(q, b)]
                                for t in range(int(K[q, b])):
                                    nc.tensor.matmul(
                                        agg_of[b][:],
                                        gt[:, bt - c0 + t, :],
                                        oh[:, :, bt - t0 + t],
                                        start=False,
                                        stop=(vis_done[b] == nvis - 1),
                                        skip_group_check=True,
                                    )
                                    vis_done[b] += 1
                        # close the round: z, MLP, emit (one chunk per round)
                        o = rblocks[0] * BLK
                        _mlp_chunk(o, (rblocks[-1] + 1) * BLK - o, agg_of)

                    RPG = GRBLK // RBLK  # rounds per gather group
                    ag_rounds = {sp // MLP_CHUNK - 1: i for i, sp in enumerate(AG_SPLITS)}
                    for step in range(NGR + PF):
                        if step < NGR:
                            _issue_round(step)
                        if step >= PF:
                            g = step - PF
                            for r in range(g * RPG, min((g + 1) * RPG, NR)):
                                _consume_round(r)
                                if l + 1 < L and r in ag_rounds:
                                    _ag_piece(l + 1, ag_rounds[r])
                    if l + 1 < L:
                        _ag_piece(l + 1, len(AG_SPLITS))

                    # extract pooled sums for this layer
                    pooled_sb = epool.tile([G, F], dt.float32, tag="pooled")
                    nc.scalar.activation(pooled_sb[:], pool_psums[l][:], AF.Copy)
                    nc.sync.dma_start(out_p[:, l * F:(l + 1) * F], pooled_sb[:])

    nc.compile()
    return nc


def _make_in_maps(struct, inputs):
    x = np.asarray(inputs["x"], dtype=_F32)
    ident = np.eye(128, dtype=_F32).astype(_BF16)

    eps_arr = np.asarray(inputs["eps"], dtype=_F32)
    epsI = np.stack([np.eye(128, dtype=_F32) * (1.0 + e) for e in eps_arr])
    shared = {
        "ident": np.ascontiguousarray(ident),
        "epsI": np.ascontiguousarray(epsI.astype(_BF16)),
        "Wp1": np.asarray(inputs["W_pre1"], dtype=_F32),
        "bp1": np.asarray(inputs["b_pre1"], dtype=_F32).reshape(F, 1),
        "Wp2": np.asarray(inputs["W_pre2"], dtype=_F32),
        "bp2": np.asarray(inputs["b_pre2"], dtype=_F32).reshape(F, 1),
        "W1s": np.asarray(inputs["W1s"], dtype=_F32),
        "b1s": np.asarray(inputs["b1s"], dtype=_F32).reshape(L, F, 1),
        "W2s": np.asarray(inputs["W2s"], dtype=_F32),
        "b2s": np.asarray(inputs["b2s"], dtype=_F32).reshape(L, F, 1),
    }

    in_maps = []
    for c in range(NC):
        xs = np.zeros((F, PERP), dtype=_F32)
        xs[:, :PER] = x[c * PER:(c + 1) * PER].T
        m = dict(shared)
        m["xT"] = xs
        m["idx"] = struct["idx_planes"][c]
        m["ohp"] = struct["oh_planes"][c]
        m["ohg"] = struct["ohg_planes"][c]
        m["iota"] = np.ascontiguousarray(
            np.broadcast_to(np.arange(128, dtype=_F32), (128, 128)).astype(_BF16))
        m["bg"] = struct["bg_planes"][c]
        in_maps.append(m)
    return in_maps


def kernel(**inputs):
    from concourse.bass_utils import run_bass_kernel_spmd

    edge_index = np.asarray(inputs["edge_index"])
    batch = np.asarray(inputs["batch"])
    eps = np.asarray(inputs["eps"], dtype=_F32)

    struct = _build_structures(edge_index, batch)
    nc = _build_program(struct, [float(e) for e in eps])
    in_maps = _make_in_maps(struct, inputs)

    res = run_bass_kernel_spmd(nc, in_maps, core_ids=list(range(NC)))
    out = np.zeros((G, L * F), dtype=_F32)
    for c in range(NC):
        out += res.results[c]["out"]
    return out

